# revision 22
# baseline (speedup 1.0000x reference)
"""Bahdanau-attention Bass kernel for Trainium2, data-parallel over batch on 8 cores.

reference math (per batch b):
  energy  = tanh(cat([dec_hidden broadcast, enc], -1) @ W_h.T)     [S, H]
  scores  = energy @ v                                             [S]
  w       = exp(scores) * mask    (softmax w/o max-sub; |scores| <= ||v||_1, exp-safe)
  a       = w / (sum(w) + eps)
  context = a @ enc                                                [D]

Split W_h = [W1 | W2] (decoder part | encoder part):
  energy = tanh(enc @ W2.T + dec_hidden @ W1.T)
The dec_proj term is per-(batch,h) and enters as the ACT bias.

Layouts per core (local batches j=0..7, S=2048, D=H=256):
  e_nat [128(s_in), (t, d)] bf16   natural tiles, SWDGE cast-load from fp32 DRAM
  eT    [128(d_in), (kc, s)] bf16  PE-transposed tiles (matmul contracts d on partitions)
  energy psum [128(h_in), s]       lhsT = W2T chunk (stationary), rhs = eT
  tanh  [128(h_in), (hc, s)] bf16  ACT, bias = dec_proj column
  scores psum [32*sc .. , 512]     M=32 col-tiled v-dot matmuls (v replicated 32x)
  w pipeline in [128(s_in), t] layout after a PE transpose; context matmuls use
  normalized weights as stationary [128,1] columns against e_nat tiles.
"""

import os
import sys
from contextlib import ExitStack

for _p in ("/opt/trn_rl_repo",):
    if _p not in sys.path and os.path.isdir(_p):
        sys.path.insert(0, _p)

import numpy as np
import ml_dtypes

import concourse.bass as bass
import concourse.bacc as bacc
import concourse.tile as tile
from concourse import masks, mybir
from concourse.bass_utils import run_bass_kernel_spmd

# problem dims (hardcoded per contract)
B, S, H, D = 64, 2048, 256, 256
NCORES = 8
BL = B // NCORES            # batches per core
P = 128                     # partitions
T = S // P                  # s tiles per batch
KC = D // P                 # contraction chunks (d)
HC = H // P                 # h chunks
EPS = 1e-10

F32 = mybir.dt.float32
BF16 = mybir.dt.bfloat16


def build_kernel(bl=BL, s=S, reps=1, stop_after=99, cfg=None):
    """Build the single-core Bass program (same program on all cores: SPMD).

    stop_after: pipeline-prefix ablation for cost attribution.
      1=load 2=transpose 3=proj+tanh 4=vdot 5=softmax 6=a-out 7=context
    """
    cfg = dict(cfg or {})
    c_xbar = cfg.get("xbar", False)
    c_psE = cfg.get("psE", 3)
    c_psT = cfg.get("psT", 2)
    c_psS = cfg.get("psS", 1)
    c_psX = cfg.get("psX", 1)
    c_sbbufs = cfg.get("sbbufs", 3)
    c_epair = cfg.get("epair", 1)   # sc chunks per psE tile (1 or 2)
    c_act_copy = cfg.get("act_copy", 0)  # fraction of eT copies on ACT (0..16)
    t_tiles = s // P
    sc_n = s // 512
    nc = bacc.Bacc("TRN2", target_bir_lowering=False, debug=False, num_devices=1)

    enc = nc.dram_tensor("enc", [bl, s, D], F32, kind="ExternalInput")
    w2t = nc.dram_tensor("w2t", [D, H], BF16, kind="ExternalInput")
    w1t = nc.dram_tensor("w1t", [H, H], F32, kind="ExternalInput")
    dht = nc.dram_tensor("dht", [H, bl], F32, kind="ExternalInput")
    v32 = nc.dram_tensor("v32", [P, HC * 32], BF16, kind="ExternalInput")
    maskc = nc.dram_tensor("maskc", [s, bl], F32, kind="ExternalInput")
    a_out = nc.dram_tensor("a_out", [bl, s], F32, kind="ExternalOutput")
    ctx_out = nc.dram_tensor("ctx_out", [bl, D], F32, kind="ExternalOutput")

    with tile.TileContext(nc) as tc, ExitStack() as ctx:
        const = ctx.enter_context(tc.tile_pool(name="const", bufs=1))
        enat_pool = ctx.enter_context(tc.tile_pool(name="enat", bufs=c_sbbufs))
        eT_pool = ctx.enter_context(tc.tile_pool(name="eT", bufs=c_sbbufs))
        tanh_pool = ctx.enter_context(tc.tile_pool(name="tanh", bufs=c_sbbufs))
        ssb_pool = ctx.enter_context(tc.tile_pool(name="ssb", bufs=2))
        small = ctx.enter_context(tc.tile_pool(name="small", bufs=3))
        outsb = ctx.enter_context(tc.tile_pool(name="outsb", bufs=1))

        psE = ctx.enter_context(tc.tile_pool(name="psE", bufs=c_psE, space="PSUM"))
        psT = ctx.enter_context(tc.tile_pool(name="psT", bufs=c_psT, space="PSUM"))
        psS = ctx.enter_context(tc.tile_pool(name="psS", bufs=c_psS, space="PSUM"))
        psX = ctx.enter_context(tc.tile_pool(name="psX", bufs=c_psX, space="PSUM"))
        psC = ctx.enter_context(tc.tile_pool(name="psC", bufs=1, space="PSUM"))

        # ---- constants / prologue ----
        ident_bf = const.tile([P, P], BF16)
        ident_f32 = const.tile([P, P], F32)
        masks.make_identity(nc, ident_bf[:])
        masks.make_identity(nc, ident_f32[:])
        ones_f32 = const.tile([P, P], F32)
        nc.gpsimd.memset(ones_f32[:], 1.0)

        w2t_sb = const.tile([P, KC * H], BF16)
        nc.sync.dma_start(
            out=w2t_sb[:].rearrange("p (kc h) -> p kc h", kc=KC),
            in_=w2t.ap().rearrange("(kc p) h -> p kc h", p=P),
        )
        w1t_sb = const.tile([P, KC * H], F32)
        nc.sync.dma_start(
            out=w1t_sb[:].rearrange("p (kc h) -> p kc h", kc=KC),
            in_=w1t.ap().rearrange("(kc p) h -> p kc h", p=P),
        )
        dht_sb = const.tile([P, KC * bl], F32)
        nc.sync.dma_start(
            out=dht_sb[:].rearrange("p (kc j) -> p kc j", kc=KC),
            in_=dht.ap().rearrange("(kc p) j -> p kc j", p=P),
        )
        v32_sb = const.tile([P, HC * 32], BF16)
        nc.sync.dma_start(out=v32_sb[:], in_=v32.ap())
        mask_sb = const.tile([P, t_tiles * bl], F32)
        nc.sync.dma_start(
            out=mask_sb[:].rearrange("p (t j) -> p t j", j=bl),
            in_=maskc.ap().rearrange("(t p) j -> p t j", p=P),
        )

        # dec_proj[h, j] = sum_k W1[h,k] dec_hidden[j,k]; psum -> sbuf
        pd = psX.tile([P, 512], F32, tag="x")
        for hc in range(HC):
            for kc in range(KC):
                nc.tensor.matmul(
                    pd[:, hc * bl : (hc + 1) * bl],
                    lhsT=w1t_sb[:, kc * H + hc * P : kc * H + (hc + 1) * P],
                    rhs=dht_sb[:, kc * bl : (kc + 1) * bl],
                    start=(kc == 0),
                    stop=(kc == KC - 1),
                )
        dec_sb = const.tile([P, HC * bl], F32)
        nc.vector.tensor_copy(dec_sb[:], pd[:, : HC * bl])

        a_sb = outsb.tile([P, bl * P], F32, name="a_sb", tag="a_sb") if stop_after >= 6 else None
        ctx_sb = outsb.tile([P, bl * D], F32, name="ctx_sb", tag="ctx_sb") if stop_after >= 7 else None

        # ---- main per-batch pipeline ----
        for _rep in range(reps):
            for j in range(bl):
                # 1) cast-load natural tiles
                enat = enat_pool.tile([P, t_tiles * D], BF16)
                nc.gpsimd.dma_start(
                    out=enat[:].rearrange("p (t d) -> p t d", d=D),
                    in_=enc.ap()[j].rearrange("(t p) d -> p t d", p=P),
                )
                if stop_after < 2:
                    continue

                # 2) transpose into eT [128(d_in), (kc, s)]
                eT = eT_pool.tile([P, KC * s], BF16)
                if c_xbar:
                    for t in range(t_tiles):
                        for dc in range(KC):
                            nc.sync.dma_start_transpose(
                                eT[:, (t * KC + dc) * P : (t * KC + dc + 1) * P],
                                enat[:, t * D + dc * P : t * D + (dc + 1) * P],
                            )
                else:
                    for tp in range(t_tiles // 2):  # pairs of t
                        pt = psT.tile([P, 512], BF16)
                        for ti in range(2):
                            t = tp * 2 + ti
                            for dc in range(KC):
                                nc.tensor.transpose(
                                    pt[:, (ti * KC + dc) * P : (ti * KC + dc + 1) * P],
                                    enat[:, t * D + dc * P : t * D + (dc + 1) * P],
                                    ident_bf[:],
                                )
                        dst = eT[:, tp * 512 : (tp + 1) * 512]
                        if tp < c_act_copy:
                            nc.scalar.copy(dst, pt[:])
                        else:
                            nc.vector.tensor_copy(dst, pt[:])
                if stop_after < 3:
                    continue

                # 3) projection matmuls + tanh(+dec bias), [128,1024] psum tiles
                tanh_t = tanh_pool.tile([P, HC * s], BF16)
                npair = max(sc_n // c_epair, 1)
                per = 512 * min(sc_n, c_epair)
                for hc in range(HC):
                    for scp in range(npair):
                        pe = psE.tile([P, per], F32)
                        for kc in range(KC):
                            for half in range(min(sc_n, c_epair)):
                                sc = scp * c_epair + half
                                nc.tensor.matmul(
                                    pe[:, half * 512 : (half + 1) * 512],
                                    lhsT=w2t_sb[:, kc * H + hc * P : kc * H + (hc + 1) * P],
                                    rhs=eT[:].rearrange(
                                        "p (t kc x) -> p kc t x", kc=KC, x=P
                                    )[:, kc, sc * 4 : (sc + 1) * 4, :],
                                    start=(kc == 0),
                                    stop=(kc == KC - 1),
                                )
                        nc.scalar.activation(
                            tanh_t[:, hc * s + scp * per : hc * s + (scp + 1) * per],
                            pe[:],
                            mybir.ActivationFunctionType.Tanh,
                            bias=dec_sb[:, hc * bl + j : hc * bl + j + 1],
                        )
                if stop_after < 4:
                    continue

                # 4) v-dot -> scores psum rows [32sc..32sc+32) (col-tiled, M=32)
                pss = psS.tile([P, 512], F32)
                for sc in range(sc_n):
                    for hc in range(HC):
                        nc.tensor.matmul(
                            pss[32 * sc : 32 * (sc + 1), :],
                            lhsT=v32_sb[:, hc * 32 : (hc + 1) * 32],
                            rhs=tanh_t[:, hc * s + sc * 512 : hc * s + (sc + 1) * 512],
                            start=(hc == 0),
                            stop=(hc == HC - 1),
                            tile_position=(0, 32 * sc),
                        )
                if stop_after < 5:
                    continue

                # 5) scores -> sbuf, transpose to [s_in, t], exp
                live = 32 * sc_n
                ssb = ssb_pool.tile([P, 512], F32)
                nc.vector.tensor_copy(ssb[0:live, :], pss[0:live, :])
                pT2 = psX.tile([P, 512], F32, tag="x")
                for tt in range(4):
                    nc.tensor.transpose(
                        pT2[:, tt * P : tt * P + live],
                        ssb[0:live, tt * P : (tt + 1) * P],
                        ident_f32[0:live, 0:live],
                    )
                expT = small.tile([P, t_tiles], F32, tag="expT")
                src = pT2[:].rearrange("p (tt q y) -> p q tt y", tt=4, y=32)[
                    :, 0:sc_n, :, 0:1
                ]
                nc.scalar.activation(
                    expT[:].rearrange("p (q tt y) -> p q tt y", q=sc_n, y=1),
                    src,
                    mybir.ActivationFunctionType.Exp,
                )

                # 6) mask, partials, total via ones-matmul, 1/(sum+eps)
                wm = small.tile([P, t_tiles], F32, tag="wm")
                partial = small.tile([P, 1], F32, tag="partial")
                nc.vector.tensor_tensor(
                    out=wm[:],
                    in0=expT[:],
                    in1=mask_sb[:, j : j + (t_tiles - 1) * bl + 1 : bl],
                    op=mybir.AluOpType.mult,
                )
                nc.vector.tensor_reduce(
                    out=partial[:],
                    in_=wm[:],
                    axis=mybir.AxisListType.X,
                    op=mybir.AluOpType.add,
                )
                psum_sum = psX.tile([P, 512], F32, tag="x")
                nc.tensor.matmul(
                    psum_sum[:, 0:1], lhsT=ones_f32[:], rhs=partial[:],
                    start=True, stop=True,
                )
                inv = small.tile([P, 1], F32, tag="inv")
                nc.vector.tensor_scalar_add(inv[:], psum_sum[:, 0:1], EPS)
                inv2 = small.tile([P, 1], F32, tag="inv2")
                nc.vector.reciprocal(inv2[:], inv[:])

                # 7) normalized weights: bf16 for context, fp32 for the a output
                wn = small.tile([P, t_tiles], BF16, tag="wn")
                nc.vector.tensor_scalar_mul(wn[:], wm[:], inv2[:])
                if stop_after >= 6:
                    wa = small.tile([P, t_tiles], F32, tag="wa")
                    nc.vector.tensor_scalar_mul(wa[:], wm[:], inv2[:])

                    # 8) a output: transpose [s_in, t] -> [t, s_in]
                    pwa = psX.tile([P, 512], F32, tag="x")
                    nc.tensor.transpose(pwa[0:t_tiles, 0:P], wa[:], ident_f32[:])
                    nc.vector.tensor_copy(
                        a_sb[0:t_tiles, j * P : (j + 1) * P], pwa[0:t_tiles, 0:P]
                    )

                # 9) context
                if stop_after >= 7:
                    pctx = psC.tile([P, 512], F32)
                    for t in range(t_tiles):
                        nc.tensor.matmul(
                            pctx[0:1, 0:D],
                            lhsT=wn[:, t : t + 1],
                            rhs=enat[:, t * D : (t + 1) * D],
                            start=(t == 0),
                            stop=(t == t_tiles - 1),
                        )
                    nc.vector.tensor_copy(
                        ctx_sb[0:1, j * D : (j + 1) * D], pctx[0:1, 0:D]
                    )

        # ---- outputs ----
        if stop_after >= 6:
            nc.sync.dma_start(
                out=a_out.ap().rearrange("j (t x) -> t j x", x=P),
                in_=a_sb[0:t_tiles, :].rearrange("t (j x) -> t j x", x=P),
            )
        if stop_after >= 7:
            nc.sync.dma_start(
                out=ctx_out.ap().rearrange("j d -> (j d)").rearrange(
                    "(o f) -> o f", o=1
                ),
                in_=ctx_sb[0:1, :],
            )
    nc.compile()
    return nc


def _host_inputs(encoder_outputs, decoder_hidden, inp_mask, W_h, v):
    """Shard + lay out host-side inputs per core."""
    enc = np.ascontiguousarray(encoder_outputs, dtype=np.float32)
    dh = np.asarray(decoder_hidden, dtype=np.float32)
    mask = np.asarray(inp_mask)
    W = np.asarray(W_h, dtype=np.float32)
    vv = np.asarray(v, dtype=np.float32)

    w1t = np.ascontiguousarray(W[:, :H].T)                      # [k, h] fp32
    w2t = np.ascontiguousarray(W[:, H:].T).astype(ml_dtypes.bfloat16)  # [d, h] bf16
    v32 = np.ascontiguousarray(
        np.repeat(vv.reshape(HC, P).T[:, :, None], 32, axis=2).reshape(P, HC * 32)
    ).astype(ml_dtypes.bfloat16)

    in_maps = []
    for c in range(NCORES):
        sl = slice(c * BL, (c + 1) * BL)
        in_maps.append(
            {
                "enc": np.ascontiguousarray(enc[sl]),
                "w2t": w2t,
                "w1t": w1t,
                "dht": np.ascontiguousarray(dh[sl].T),
                "v32": v32,
                "maskc": np.ascontiguousarray(mask[:, sl]).astype(np.float32),
            }
        )
    return in_maps


_NC_CACHE = {}


def kernel(encoder_outputs, decoder_hidden, inp_mask, W_h, v):
    if "nc" not in _NC_CACHE:
        _NC_CACHE["nc"] = build_kernel()
    nc = _NC_CACHE["nc"]
    in_maps = _host_inputs(encoder_outputs, decoder_hidden, inp_mask, W_h, v)
    res = run_bass_kernel_spmd(nc, in_maps, list(range(NCORES)))
    a = np.empty((B, 1, S), dtype=np.float32)
    context = np.empty((B, 1, D), dtype=np.float32)
    for c in range(NCORES):
        out = res.results[c]
        a[c * BL : (c + 1) * BL, 0, :] = out["a_out"]
        context[c * BL : (c + 1) * BL, 0, :] = out["ctx_out"]
    return a, context


# revision 24
# speedup vs baseline: 1.5628x; 1.5628x over previous
"""Bahdanau-attention Bass kernel for Trainium2, data-parallel over batch on 8 cores.

reference math (per batch b):
  energy  = tanh(cat([dec_hidden broadcast, enc], -1) @ W_h.T)     [S, H]
  scores  = energy @ v                                             [S]
  w       = exp(scores) * mask    (softmax w/o max-sub; |scores| <= ||v||_1, exp-safe)
  a       = w / (sum(w) + eps)
  context = a @ enc                                                [D]

Split W_h = [W1 | W2] (decoder part | encoder part):
  energy = tanh(enc @ W2.T + dec_hidden @ W1.T)
The dec_proj term is per-(batch,h) and enters as the ACT bias.

Layouts per core (local batches j=0..7, S=2048, D=H=256):
  e_nat [128(s_in), (t, d)] bf16   natural tiles, SWDGE cast-load from fp32 DRAM
  eT    [128(d_in), (kc, s)] bf16  PE-transposed tiles (matmul contracts d on partitions)
  energy psum [128(h_in), s]       lhsT = W2T chunk (stationary), rhs = eT
  tanh  [128(h_in), (hc, s)] bf16  ACT, bias = dec_proj column
  scores psum [32*sc .. , 512]     M=32 col-tiled v-dot matmuls (v replicated 32x)
  w pipeline in [128(s_in), t] layout after a PE transpose; context matmuls use
  normalized weights as stationary [128,1] columns against e_nat tiles.
"""

import os
import sys
from contextlib import ExitStack

for _p in ("/opt/trn_rl_repo",):
    if _p not in sys.path and os.path.isdir(_p):
        sys.path.insert(0, _p)

import numpy as np
import ml_dtypes

import concourse.bass as bass
import concourse.bacc as bacc
import concourse.tile as tile
from concourse import masks, mybir
from concourse.bass_utils import run_bass_kernel_spmd

# problem dims (hardcoded per contract)
B, S, H, D = 64, 2048, 256, 256
NCORES = 8
BL = B // NCORES            # batches per core
P = 128                     # partitions
T = S // P                  # s tiles per batch
KC = D // P                 # contraction chunks (d)
HC = H // P                 # h chunks
EPS = 1e-10

F32 = mybir.dt.float32
BF16 = mybir.dt.bfloat16


def build_kernel(bl=BL, s=S, reps=1, stop_after=99, cfg=None):
    """Build the single-core Bass program (same program on all cores: SPMD).

    stop_after: pipeline-prefix ablation for cost attribution.
      1=load 2=transpose 3=proj+tanh 4=vdot 5=softmax 6=a-out 7=context
    """
    cfg = dict(cfg or {})
    c_xbar = cfg.get("xbar", False)
    c_psE = cfg.get("psE", 3)
    c_psT = cfg.get("psT", 2)
    c_psS = cfg.get("psS", 1)
    c_psX = cfg.get("psX", 1)
    c_sbbufs = cfg.get("sbbufs", 3)
    c_epair = cfg.get("epair", 1)   # sc chunks per psE tile (1 or 2)
    c_act_copy = cfg.get("act_copy", 0)  # fraction of eT copies on ACT (0..16)
    c_vdot_il = cfg.get("vdot_il", False)   # interleave vdot col groups (HW concurrency)
    c_ctx_grp = cfg.get("ctx_grp", False)   # col-tile context across batch groups of 4
    t_tiles = s // P
    sc_n = s // 512
    nc = bacc.Bacc("TRN2", target_bir_lowering=False, debug=False, num_devices=1)

    enc = nc.dram_tensor("enc", [bl, s, D], F32, kind="ExternalInput")
    w2t = nc.dram_tensor("w2t", [D, H], BF16, kind="ExternalInput")
    w1t = nc.dram_tensor("w1t", [H, H], F32, kind="ExternalInput")
    dht = nc.dram_tensor("dht", [H, bl], F32, kind="ExternalInput")
    v32 = nc.dram_tensor("v32", [P, HC * 32], BF16, kind="ExternalInput")
    maskc = nc.dram_tensor("maskc", [s, bl], F32, kind="ExternalInput")
    a_out = nc.dram_tensor("a_out", [bl, s], F32, kind="ExternalOutput")
    ctx_out = nc.dram_tensor("ctx_out", [bl, D], F32, kind="ExternalOutput")

    with tile.TileContext(nc) as tc, ExitStack() as ctx:
        const = ctx.enter_context(tc.tile_pool(name="const", bufs=1))
        enat_pool = ctx.enter_context(tc.tile_pool(name="enat", bufs=c_sbbufs))
        eT_pool = ctx.enter_context(tc.tile_pool(name="eT", bufs=c_sbbufs))
        tanh_pool = ctx.enter_context(tc.tile_pool(name="tanh", bufs=c_sbbufs))
        ssb_pool = ctx.enter_context(tc.tile_pool(name="ssb", bufs=2))
        small = ctx.enter_context(tc.tile_pool(name="small", bufs=6))
        outsb = ctx.enter_context(tc.tile_pool(name="outsb", bufs=1))

        psE = ctx.enter_context(tc.tile_pool(name="psE", bufs=c_psE, space="PSUM"))
        psT = ctx.enter_context(tc.tile_pool(name="psT", bufs=c_psT, space="PSUM"))
        psS = ctx.enter_context(tc.tile_pool(name="psS", bufs=c_psS, space="PSUM"))
        psX = ctx.enter_context(tc.tile_pool(name="psX", bufs=c_psX, space="PSUM"))
        psC = ctx.enter_context(tc.tile_pool(name="psC", bufs=1, space="PSUM"))

        # ---- constants / prologue ----
        ident_bf = const.tile([P, P], BF16)
        ident_f32 = const.tile([P, P], F32)
        masks.make_identity(nc, ident_bf[:])
        masks.make_identity(nc, ident_f32[:])
        ones_f32 = const.tile([P, P], F32)
        nc.gpsimd.memset(ones_f32[:], 1.0)

        w2t_sb = const.tile([P, KC * H], BF16)
        nc.sync.dma_start(
            out=w2t_sb[:].rearrange("p (kc h) -> p kc h", kc=KC),
            in_=w2t.ap().rearrange("(kc p) h -> p kc h", p=P),
        )
        w1t_sb = const.tile([P, KC * H], F32)
        nc.sync.dma_start(
            out=w1t_sb[:].rearrange("p (kc h) -> p kc h", kc=KC),
            in_=w1t.ap().rearrange("(kc p) h -> p kc h", p=P),
        )
        dht_sb = const.tile([P, KC * bl], F32)
        nc.sync.dma_start(
            out=dht_sb[:].rearrange("p (kc j) -> p kc j", kc=KC),
            in_=dht.ap().rearrange("(kc p) j -> p kc j", p=P),
        )
        v32_sb = const.tile([P, HC * 32], BF16)
        nc.sync.dma_start(out=v32_sb[:], in_=v32.ap())
        mask_sb = const.tile([P, t_tiles * bl], F32)
        nc.sync.dma_start(
            out=mask_sb[:].rearrange("p (t j) -> p t j", j=bl),
            in_=maskc.ap().rearrange("(t p) j -> p t j", p=P),
        )

        # dec_proj[h, j] = sum_k W1[h,k] dec_hidden[j,k]; psum -> sbuf
        pd = psX.tile([P, 512], F32, tag="x")
        for hc in range(HC):
            for kc in range(KC):
                nc.tensor.matmul(
                    pd[:, hc * bl : (hc + 1) * bl],
                    lhsT=w1t_sb[:, kc * H + hc * P : kc * H + (hc + 1) * P],
                    rhs=dht_sb[:, kc * bl : (kc + 1) * bl],
                    start=(kc == 0),
                    stop=(kc == KC - 1),
                )
        dec_sb = const.tile([P, HC * bl], F32)
        nc.vector.tensor_copy(dec_sb[:], pd[:, : HC * bl])

        a_sb = outsb.tile([P, bl * P], F32, name="a_sb", tag="a_sb") if stop_after >= 6 else None
        ctx_sb = outsb.tile([P, bl * D], F32, name="ctx_sb", tag="ctx_sb") if stop_after >= 7 else None

        # ---- main per-batch pipeline ----
        grp = {}
        for _rep in range(reps):
            for j in range(bl):
                # 1) cast-load natural tiles
                enat = enat_pool.tile([P, t_tiles * D], BF16)
                nc.gpsimd.dma_start(
                    out=enat[:].rearrange("p (t d) -> p t d", d=D),
                    in_=enc.ap()[j].rearrange("(t p) d -> p t d", p=P),
                )
                if stop_after < 2:
                    continue

                # 2) transpose into eT [128(d_in), (kc, s)]
                eT = eT_pool.tile([P, KC * s], BF16)
                if c_xbar:
                    for t in range(t_tiles):
                        for dc in range(KC):
                            nc.sync.dma_start_transpose(
                                eT[:, (t * KC + dc) * P : (t * KC + dc + 1) * P],
                                enat[:, t * D + dc * P : t * D + (dc + 1) * P],
                            )
                else:
                    for tp in range(t_tiles // 2):  # pairs of t
                        pt = psT.tile([P, 512], BF16)
                        for ti in range(2):
                            t = tp * 2 + ti
                            for dc in range(KC):
                                nc.tensor.transpose(
                                    pt[:, (ti * KC + dc) * P : (ti * KC + dc + 1) * P],
                                    enat[:, t * D + dc * P : t * D + (dc + 1) * P],
                                    ident_bf[:],
                                )
                        dst = eT[:, tp * 512 : (tp + 1) * 512]
                        if tp < c_act_copy:
                            nc.scalar.copy(dst, pt[:])
                        else:
                            nc.vector.tensor_copy(dst, pt[:])
                if stop_after < 3:
                    continue

                # 3) projection matmuls + tanh(+dec bias), [128,1024] psum tiles
                tanh_t = tanh_pool.tile([P, HC * s], BF16)
                npair = max(sc_n // c_epair, 1)
                per = 512 * min(sc_n, c_epair)
                for hc in range(HC):
                    for scp in range(npair):
                        pe = psE.tile([P, per], F32)
                        for kc in range(KC):
                            for half in range(min(sc_n, c_epair)):
                                sc = scp * c_epair + half
                                nc.tensor.matmul(
                                    pe[:, half * 512 : (half + 1) * 512],
                                    lhsT=w2t_sb[:, kc * H + hc * P : kc * H + (hc + 1) * P],
                                    rhs=eT[:].rearrange(
                                        "p (t kc x) -> p kc t x", kc=KC, x=P
                                    )[:, kc, sc * 4 : (sc + 1) * 4, :],
                                    start=(kc == 0),
                                    stop=(kc == KC - 1),
                                )
                        nc.scalar.activation(
                            tanh_t[:, hc * s + scp * per : hc * s + (scp + 1) * per],
                            pe[:],
                            mybir.ActivationFunctionType.Tanh,
                            bias=dec_sb[:, hc * bl + j : hc * bl + j + 1],
                        )
                if stop_after < 4:
                    continue

                # 4) v-dot -> scores psum rows [32sc..32sc+32) (col-tiled, M=32)
                pss = psS.tile([P, 512], F32)
                vorder = (
                    [(sc, hc) for hc in range(HC) for sc in range(sc_n)]
                    if c_vdot_il
                    else [(sc, hc) for sc in range(sc_n) for hc in range(HC)]
                )
                for sc, hc in vorder:
                    nc.tensor.matmul(
                        pss[32 * sc : 32 * (sc + 1), :],
                        lhsT=v32_sb[:, hc * 32 : (hc + 1) * 32],
                        rhs=tanh_t[:, hc * s + sc * 512 : hc * s + (sc + 1) * 512],
                        start=(hc == 0),
                        stop=(hc == HC - 1),
                        tile_position=(0, 32 * sc),
                        skip_group_check=c_vdot_il,
                    )
                if stop_after < 5:
                    continue

                # 5) scores -> sbuf, transpose to [s_in, t], exp
                live = 32 * sc_n
                ssb = ssb_pool.tile([P, 512], F32)
                nc.vector.tensor_copy(ssb[0:live, :], pss[0:live, :])
                pT2 = psX.tile([P, 512], F32, tag="x")
                for tt in range(4):
                    nc.tensor.transpose(
                        pT2[:, tt * P : tt * P + live],
                        ssb[0:live, tt * P : (tt + 1) * P],
                        ident_f32[0:live, 0:live],
                    )
                expT = small.tile([P, t_tiles], F32, tag="expT")
                src = pT2[:].rearrange("p (tt q y) -> p q tt y", tt=4, y=32)[
                    :, 0:sc_n, :, 0:1
                ]
                nc.scalar.activation(
                    expT[:].rearrange("p (q tt y) -> p q tt y", q=sc_n, y=1),
                    src,
                    mybir.ActivationFunctionType.Exp,
                )

                # 6) mask, partials, total via ones-matmul, 1/(sum+eps)
                wm = small.tile([P, t_tiles], F32, tag="wm")
                partial = small.tile([P, 1], F32, tag="partial")
                nc.vector.tensor_tensor(
                    out=wm[:],
                    in0=expT[:],
                    in1=mask_sb[:, j : j + (t_tiles - 1) * bl + 1 : bl],
                    op=mybir.AluOpType.mult,
                )
                nc.vector.tensor_reduce(
                    out=partial[:],
                    in_=wm[:],
                    axis=mybir.AxisListType.X,
                    op=mybir.AluOpType.add,
                )
                psum_sum = psX.tile([P, 512], F32, tag="x")
                nc.tensor.matmul(
                    psum_sum[:, 0:1], lhsT=ones_f32[:], rhs=partial[:],
                    start=True, stop=True,
                )
                inv = small.tile([P, 1], F32, tag="inv")
                nc.vector.tensor_scalar_add(inv[:], psum_sum[:, 0:1], EPS)
                inv2 = small.tile([P, 1], F32, tag="inv2")
                nc.vector.reciprocal(inv2[:], inv[:])

                # 7) normalized weights: bf16 for context, fp32 for the a output
                wn = small.tile([P, t_tiles], BF16, tag="wn")
                nc.vector.tensor_scalar_mul(wn[:], wm[:], inv2[:])
                if stop_after >= 6:
                    wa = small.tile([P, t_tiles], F32, tag="wa")
                    nc.vector.tensor_scalar_mul(wa[:], wm[:], inv2[:])

                    # 8) a output: transpose [s_in, t] -> [t, s_in]
                    pwa = psX.tile([P, 512], F32, tag="x")
                    nc.tensor.transpose(pwa[0:t_tiles, 0:P], wa[:], ident_f32[:])
                    nc.vector.tensor_copy(
                        a_sb[0:t_tiles, j * P : (j + 1) * P], pwa[0:t_tiles, 0:P]
                    )

                # 9) context
                if stop_after >= 7 and not c_ctx_grp:
                    pctx = psC.tile([P, 512], F32)
                    for t in range(t_tiles):
                        nc.tensor.matmul(
                            pctx[0:1, 0:D],
                            lhsT=wn[:, t : t + 1],
                            rhs=enat[:, t * D : (t + 1) * D],
                            start=(t == 0),
                            stop=(t == t_tiles - 1),
                        )
                    nc.vector.tensor_copy(
                        ctx_sb[0:1, j * D : (j + 1) * D], pctx[0:1, 0:D]
                    )
                if stop_after >= 7 and c_ctx_grp:
                    grp[j % 4] = (wn, enat)
                    if j % 4 == 3:
                        pctx = psC.tile([P, 512], F32)
                        for t in range(t_tiles):
                            for jj in range(4):
                                wn_j, enat_j = grp[jj]
                                nc.tensor.matmul(
                                    pctx[32 * jj : 32 * jj + 1, 0:D],
                                    lhsT=wn_j[:, t : t + 1],
                                    rhs=enat_j[:, t * D : (t + 1) * D],
                                    start=(t == 0),
                                    stop=(t == t_tiles - 1),
                                    tile_position=(0, 32 * jj),
                                    skip_group_check=True,
                                )
                        for jj in range(4):
                            nc.vector.tensor_copy(
                                ctx_sb[0:1, (j - 3 + jj) * D : (j - 2 + jj) * D],
                                pctx[32 * jj : 32 * jj + 1, 0:D],
                            )

        # ---- outputs ----
        if stop_after >= 6:
            nc.sync.dma_start(
                out=a_out.ap().rearrange("j (t x) -> t j x", x=P),
                in_=a_sb[0:t_tiles, :].rearrange("t (j x) -> t j x", x=P),
            )
        if stop_after >= 7:
            nc.sync.dma_start(
                out=ctx_out.ap().rearrange("j d -> (j d)").rearrange(
                    "(o f) -> o f", o=1
                ),
                in_=ctx_sb[0:1, :],
            )
    nc.compile()
    return nc


def _host_inputs(encoder_outputs, decoder_hidden, inp_mask, W_h, v):
    """Shard + lay out host-side inputs per core."""
    enc = np.ascontiguousarray(encoder_outputs, dtype=np.float32)
    dh = np.asarray(decoder_hidden, dtype=np.float32)
    mask = np.asarray(inp_mask)
    W = np.asarray(W_h, dtype=np.float32)
    vv = np.asarray(v, dtype=np.float32)

    w1t = np.ascontiguousarray(W[:, :H].T)                      # [k, h] fp32
    w2t = np.ascontiguousarray(W[:, H:].T).astype(ml_dtypes.bfloat16)  # [d, h] bf16
    v32 = np.ascontiguousarray(
        np.repeat(vv.reshape(HC, P).T[:, :, None], 32, axis=2).reshape(P, HC * 32)
    ).astype(ml_dtypes.bfloat16)

    in_maps = []
    for c in range(NCORES):
        sl = slice(c * BL, (c + 1) * BL)
        in_maps.append(
            {
                "enc": np.ascontiguousarray(enc[sl]),
                "w2t": w2t,
                "w1t": w1t,
                "dht": np.ascontiguousarray(dh[sl].T),
                "v32": v32,
                "maskc": np.ascontiguousarray(mask[:, sl]).astype(np.float32),
            }
        )
    return in_maps


_NC_CACHE = {}


def kernel(encoder_outputs, decoder_hidden, inp_mask, W_h, v):
    if "nc" not in _NC_CACHE:
        _NC_CACHE["nc"] = build_kernel()
    nc = _NC_CACHE["nc"]
    in_maps = _host_inputs(encoder_outputs, decoder_hidden, inp_mask, W_h, v)
    res = run_bass_kernel_spmd(nc, in_maps, list(range(NCORES)))
    a = np.empty((B, 1, S), dtype=np.float32)
    context = np.empty((B, 1, D), dtype=np.float32)
    for c in range(NCORES):
        out = res.results[c]
        a[c * BL : (c + 1) * BL, 0, :] = out["a_out"]
        context[c * BL : (c + 1) * BL, 0, :] = out["ctx_out"]
    return a, context
